# revision 21
# baseline (speedup 1.0000x reference)
"""Multi-head attention (B=2,S=2048,D=512,H=8,depth=64) + causal-mask softmax
+ output projection + residual + LayerNorm, returning (out, attn).

Sharding: sequence-parallel over query rows, causal-load-balanced. 8 cores;
core c handles batch b = c // 4 and the four global 128-row q-tiles
{j, 7-j, 8+j, 15-j} (j = c % 4). With a causal mask, local q-tile t then
has exactly t+1 live 512-wide k-chunks on EVERY core, so the SPMD program
is uniform while skipping the fully-masked upper-triangular blocks.
A dense variant (all chunks live, additive mask everywhere) is built when
the host detects the mask is not exactly causal.

Per-core device pipeline:
  - host uploads qT [D,512](transposed q rows), qn (= q rows + bo), kT, vT,
    mask chunks (bf16), weights, biases, ln params.
  - Q^T = wq^T @ qT, K^T = wk^T @ kT (bias fused into the PSUM->SBUF copy
    as a per-partition tensor_scalar add), V = vT_tiles^T @ wv (bias via
    broadcast tensor_add in the copy).
  - logits psum = Q_h^T.T @ K_h^T (fp32) + bf16 identity-matmul of
    (-79872*mask) for masked chunks; exp on ScalarE (scale=1/8) with
    accum_out row-sums; reciprocal; normalize in-place (tensor_scalar).
  - attn output DMA (plus a zero-tile DMA for skipped chunks).
  - P^T via PE transposes; PV matmul -> normalized ctx^T [64, q] per head.
  - O = ctx^T.T @ wo + residual(qn) then LayerNorm via bn_stats -> out DMA.
"""

import numpy as np
import ml_dtypes

import concourse.bacc as bacc
import concourse.tile as tile
from concourse import mybir
from concourse.masks import make_identity
from concourse.bass_utils import run_bass_kernel_spmd

B, S, D, H, DEPTH = 2, 2048, 512, 8, 64
QB = 512               # query rows per core
NQT = QB // 128        # local q-tiles per core
NCORES = 8
FP = mybir.dt.float32
BF = mybir.dt.bfloat16
MASK_SCALE = -80000.0  # ~ -10000 * 8 (exp applies scale=1/8); bf16-rounded
EPS = 1e-6

TRACE = False          # test.py sets this for profiled runs


def _build_nc(causal: bool):
    nc = bacc.Bacc(None, target_bir_lowering=False)

    # ---- I/O ----
    d = {}
    d["qT"] = nc.dram_tensor("qT", [D, QB], FP, kind="ExternalInput")
    d["qn"] = nc.dram_tensor("qn", [QB, D], FP, kind="ExternalInput")
    d["kT"] = nc.dram_tensor("kT", [D, S], FP, kind="ExternalInput")
    d["vT"] = nc.dram_tensor("vT", [D, S], FP, kind="ExternalInput")
    if causal:
        # per local q-tile: only the diagonal 512-chunk of the mask
        d["mk"] = nc.dram_tensor("mk", [NQT, 128, 512], BF,
                                 kind="ExternalInput")
    else:
        d["mk"] = nc.dram_tensor("mk", [QB, S], BF, kind="ExternalInput")
    for w in ("wq", "wk", "wv", "wo"):
        d[w] = nc.dram_tensor(w, [D, D], FP, kind="ExternalInput")
    for bnm in ("bq", "bk", "bv", "ga", "be"):
        d[bnm] = nc.dram_tensor(bnm, [D], FP, kind="ExternalInput")
    d["attn_o"] = nc.dram_tensor("attn_o", [H, QB, S], FP,
                                 kind="ExternalOutput")
    d["out_o"] = nc.dram_tensor("out_o", [QB, D], FP, kind="ExternalOutput")

    with tile.TileContext(nc) as tc:
        _emit(nc, tc, d, causal)
    nc.compile()
    return nc


def _emit(nc, tc, d, causal):
    from contextlib import ExitStack

    es = ExitStack()
    with es:
        consts = es.enter_context(tc.tile_pool(name="consts", bufs=1))
        persist = es.enter_context(tc.tile_pool(name="persist", bufs=1))
        wo_pool = es.enter_context(tc.tile_pool(name="wo", bufs=1))

        # ---- constants ----
        ident = consts.tile([128, 128], FP)
        make_identity(nc, ident)
        i80k = consts.tile([128, 128], BF)
        make_identity(nc, i80k)
        nc.vector.tensor_scalar_mul(i80k, i80k, MASK_SCALE)
        eps_t = consts.tile([128, 1], FP)
        nc.vector.memset(eps_t, EPS)
        # bias columns [128, 4]: bq/bk rearranged so tile t's per-partition
        # bias column is bqc[:, t]
        bqc = consts.tile([128, 4], FP)
        bkc = consts.tile([128, 4], FP)
        nc.sync.dma_start(out=bqc, in_=d["bq"][:].rearrange("(t p) -> p t",
                                                            p=128))
        nc.sync.dma_start(out=bkc, in_=d["bk"][:].rearrange("(t p) -> p t",
                                                            p=128))
        bv_b = consts.tile([128, D], FP)
        ga_b = consts.tile([128, D], FP)
        be_b = consts.tile([128, D], FP)
        if causal:
            zerot = consts.tile([128, 1536], FP)
            nc.vector.memset(zerot, 0.0)

        # ---- persistent SBUF ----
        KT = [persist.tile([128, S], FP, tag=f"KT{i}", name=f"KT{i}")
              for i in range(4)]
        Vt = [persist.tile([128, D], FP, tag=f"V{i}", name=f"V{i}")
              for i in range(16)]
        QT = [persist.tile([128, QB], FP, tag=f"QT{i}", name=f"QT{i}")
              for i in range(4)]
        QN = [persist.tile([128, D], FP, tag=f"QN{i}", name=f"QN{i}")
              for i in range(4)]
        CT = [persist.tile([128, QB], FP, tag=f"CT{i}", name=f"CT{i}")
              for i in range(4)]
        if causal:
            MB = [persist.tile([128, 512], BF, tag=f"MB{i}", name=f"MB{i}")
                  for i in range(4)]
        else:
            MB = [persist.tile([128, S], BF, tag=f"MB{i}", name=f"MB{i}")
                  for i in range(4)]
        wo_s = [wo_pool.tile([128, D], FP, tag=f"wo{i}", name=f"wos{i}")
                for i in range(4)]

        # ---- phase A: projections (scoped pools) ----
        with (
            tc.tile_pool(name="wqkv", bufs=1) as wqkv,
            tc.tile_pool(name="instream", bufs=4) as instream,
            tc.tile_pool(name="pA", bufs=6, space="PSUM") as pA,
        ):
            wq_s = [wqkv.tile([128, D], FP, tag=f"wq{i}", name=f"wqs{i}")
                    for i in range(4)]
            wk_s = [wqkv.tile([128, D], FP, tag=f"wk{i}", name=f"wks{i}")
                    for i in range(4)]
            wv_s = [wqkv.tile([128, D], FP, tag=f"wv{i}", name=f"wvs{i}")
                    for i in range(4)]
            wq_r = d["wq"][:].rearrange("(t p) d -> t p d", p=128)
            wk_r = d["wk"][:].rearrange("(t p) d -> t p d", p=128)
            wv_r = d["wv"][:].rearrange("(t p) d -> t p d", p=128)

            # Q^T projection (bias fused into copy); its inputs are DMA'd
            # first so the PE unblocks as early as possible
            qtin = [instream.tile([128, QB], FP, tag="qtin", name="qtin")
                    for _ in range(4)]
            for din in range(4):
                nc.sync.dma_start(out=wq_s[din], in_=wq_r[din])
                nc.sync.dma_start(
                    out=qtin[din], in_=d["qT"][din * 128:(din + 1) * 128, :])
            ps_q = [pA.tile([128, QB], FP, tag="pa", name="psq")
                    for _ in range(4)]
            for dout in range(4):
                for din in range(4):
                    nc.tensor.matmul(
                        ps_q[dout], wq_s[din][:, dout * 128:(dout + 1) * 128],
                        qtin[din], start=(din == 0), stop=(din == 3))
                nc.vector.tensor_scalar(
                    out=QT[dout], in0=ps_q[dout],
                    scalar1=bqc[:, dout:dout + 1], scalar2=None,
                    op0=mybir.AluOpType.add)

            # K^T projection, sk in chunks of 512 (bias fused into copy)
            for i in range(4):
                nc.sync.dma_start(out=wk_s[i], in_=wk_r[i])
            for skc in range(4):
                ktin = [instream.tile([128, 512], FP, tag="ktin",
                                      name="ktin", bufs=6) for _ in range(4)]
                for din in range(4):
                    nc.sync.dma_start(
                        out=ktin[din],
                        in_=d["kT"][din * 128:(din + 1) * 128,
                                    skc * 512:(skc + 1) * 512])
                ps_k = [pA.tile([128, 512], FP, tag="pa", name="psk")
                        for _ in range(4)]
                for dout in range(4):
                    for din in range(4):
                        nc.tensor.matmul(
                            ps_k[dout],
                            wk_s[din][:, dout * 128:(dout + 1) * 128],
                            ktin[din], start=(din == 0), stop=(din == 3))
                    nc.vector.tensor_scalar(
                        out=KT[dout][:, skc * 512:(skc + 1) * 512],
                        in0=ps_k[dout], scalar1=bkc[:, dout:dout + 1],
                        scalar2=None, op0=mybir.AluOpType.add)

            for i in range(4):
                if causal:
                    nc.sync.dma_start(out=MB[i], in_=d["mk"][i])
                else:
                    nc.sync.dma_start(out=MB[i],
                                      in_=d["mk"][i * 128:(i + 1) * 128, :])
                nc.sync.dma_start(out=QN[i],
                                  in_=d["qn"][i * 128:(i + 1) * 128, :])
            nc.sync.dma_start(out=bv_b, in_=d["bv"][None, :].to_broadcast(
                [128, D]))
            for i in range(4):
                nc.sync.dma_start(out=wv_s[i], in_=wv_r[i])

            # V projection (bias via broadcast add in the copy)
            for skc in range(4):
                vtin = [instream.tile([128, 512], FP, tag="vtin",
                                      name="vtin", bufs=6) for _ in range(4)]
                for din in range(4):
                    nc.sync.dma_start(
                        out=vtin[din],
                        in_=d["vT"][din * 128:(din + 1) * 128,
                                    skc * 512:(skc + 1) * 512])
                for st in range(4):
                    sk = skc * 4 + st
                    ps_v = pA.tile([128, D], FP, tag="pa", name="psv")
                    for din in range(4):
                        nc.tensor.matmul(
                            ps_v, vtin[din][:, st * 128:(st + 1) * 128],
                            wv_s[din], start=(din == 0), stop=(din == 3))
                    nc.vector.tensor_add(Vt[sk], ps_v, bv_b)

        wo_r2 = d["wo"][:].rearrange("(t p) d -> t p d", p=128)
        for i in range(4):
            nc.sync.dma_start(out=wo_s[i], in_=wo_r2[i])
        nc.sync.dma_start(out=ga_b, in_=d["ga"][None, :].to_broadcast(
            [128, D]))
        nc.sync.dma_start(out=be_b, in_=d["be"][None, :].to_broadcast(
            [128, D]))

        # ---- hot phase ----
        with (
            tc.tile_pool(name="Pp", bufs=3) as Pp,
            tc.tile_pool(name="PTp", bufs=3) as PTp,
            tc.tile_pool(name="sp", bufs=8) as sp,
            tc.tile_pool(name="Osb", bufs=2) as Osb,
            tc.tile_pool(name="Lp", bufs=2, space="PSUM") as Lp,
            tc.tile_pool(name="Tp", bufs=2, space="PSUM") as Tp,
            tc.tile_pool(name="Cp", bufs=2, space="PSUM") as Cp,
        ):
            for qt in range(4):
                for hpair in range(4):
                  PTs = []
                  for h in (2 * hpair, 2 * hpair + 1):
                    ht = h // 2
                    hp = (h % 2) * 64
                    nch = (qt + 1) if causal else 4   # live 512-chunks
                    live = nch * 512
                    P_t = Pp.tile([128, S], FP, tag="p", name="P_t")
                    s_parts = []
                    # logits psum in halves of up to 1024 cols
                    for h0 in range(0, nch, 2):
                        hw = min(2, nch - h0) * 512
                        L_t = Lp.tile([128, 1024], FP, tag="L", name="L_t")
                        for sc in range(h0, min(h0 + 2, nch)):
                            sl = slice((sc - h0) * 512, (sc - h0 + 1) * 512)
                            nc.tensor.matmul(
                                L_t[:, sl],
                                QT[ht][hp:hp + 64, qt * 128:(qt + 1) * 128],
                                KT[ht][hp:hp + 64, sc * 512:(sc + 1) * 512],
                                start=True,
                                stop=(causal and sc != qt))
                            if causal:
                                if sc == qt:
                                    nc.tensor.matmul(
                                        L_t[:, sl], i80k, MB[qt],
                                        start=False, stop=True)
                            else:
                                nc.tensor.matmul(
                                    L_t[:, sl], i80k,
                                    MB[qt][:, sc * 512:(sc + 1) * 512],
                                    start=False, stop=True)
                        sh = sp.tile([128, 1], FP, tag="sh", name="sh")
                        nc.scalar.activation(
                            out=P_t[:, h0 * 512:h0 * 512 + hw],
                            in_=L_t[:, :hw],
                            func=mybir.ActivationFunctionType.Exp,
                            scale=0.125, accum_out=sh)
                        s_parts.append(sh)
                    rs = sp.tile([128, 1], FP, tag="rs", name="rs")
                    if len(s_parts) == 1:
                        nc.vector.reciprocal(rs, s_parts[0])
                    else:
                        nc.vector.tensor_add(rs, s_parts[0], s_parts[1])
                        nc.vector.reciprocal(rs, rs)
                    nc.vector.tensor_scalar_mul(
                        P_t[:, :live], P_t[:, :live], rs)
                    nc.sync.dma_start(
                        out=d["attn_o"][h, qt * 128:(qt + 1) * 128, :live],
                        in_=P_t[:, :live])
                    if causal and live < S:
                        nc.sync.dma_start(
                            out=d["attn_o"][h, qt * 128:(qt + 1) * 128,
                                            live:],
                            in_=zerot[:, :S - live])
                    # transpose live 128-blocks -> PT
                    PT_t = PTp.tile([128, S], FP, tag="pt", name="PT_t")
                    nkt = nch * 4
                    for jg in range((nkt + 3) // 4):
                        T_t = Tp.tile([128, 512], FP, tag="tp", name="T_t")
                        n_in_g = min(4, nkt - jg * 4)
                        for jj in range(n_in_g):
                            j = jg * 4 + jj
                            nc.tensor.transpose(
                                T_t[:, jj * 128:(jj + 1) * 128],
                                P_t[:, j * 128:(j + 1) * 128], ident)
                        dst = PT_t[:, jg * 512:jg * 512 + n_in_g * 128]
                        if jg % 2 == 0:
                            nc.vector.tensor_copy(dst, T_t[:, :n_in_g * 128])
                        else:
                            nc.scalar.copy(dst, T_t[:, :n_in_g * 128])
                    PTs.append(PT_t)
                  # PV for the head pair, col-packed: the two 64-row ctx^T
                  # chains go to psum partitions 0-63 / 64-127 (distinct PE
                  # col-groups) with interleaved adjacent matmuls so the HW
                  # can run them concurrently.
                  nkt = ((qt + 1) if causal else 4) * 4
                  ct2 = Cp.tile([128, 128], FP, tag="ctx", name="ct2")
                  for j in range(nkt):
                      nc.tensor.matmul(
                          ct2[0:64, :],
                          Vt[j][:, (2 * hpair) * 64:(2 * hpair) * 64 + 64],
                          PTs[0][:, j * 128:(j + 1) * 128],
                          start=(j == 0), stop=(j == nkt - 1),
                          tile_position=(0, 0))
                      nc.tensor.matmul(
                          ct2[64:128, :],
                          Vt[j][:, (2 * hpair + 1) * 64:
                                (2 * hpair + 1) * 64 + 64],
                          PTs[1][:, j * 128:(j + 1) * 128],
                          start=(j == 0), stop=(j == nkt - 1),
                          tile_position=(0, 64))
                  nc.vector.tensor_copy(
                      CT[hpair][:, qt * 128:(qt + 1) * 128], ct2)

                # ---- output projection + residual + LayerNorm for qt ----
                O_ps = Tp.tile([128, 512], FP, tag="tp", name="O_ps")
                for din in range(4):
                    nc.tensor.matmul(
                        O_ps, CT[din][:, qt * 128:(qt + 1) * 128],
                        wo_s[din], start=(din == 0), stop=(din == 3))
                O_sb = Osb.tile([128, D], FP, tag="osb", name="O_sb")
                nc.vector.tensor_add(O_sb, O_ps, QN[qt])
                stats = sp.tile([128, 6], FP, tag="st", name="stats")
                mv = sp.tile([128, 2], FP, tag="mv", name="mv")
                nc.vector.bn_stats(stats, O_sb)
                nc.vector.bn_aggr(mv, stats)
                rstd = sp.tile([128, 1], FP, tag="rstd", name="rstd")
                nc.scalar.activation(
                    out=rstd, in_=mv[:, 1:2],
                    func=mybir.ActivationFunctionType.Sqrt, bias=eps_t)
                nc.vector.reciprocal(rstd, rstd)
                nc.vector.tensor_scalar(
                    out=O_sb, in0=O_sb, scalar1=mv[:, 0:1], scalar2=rstd,
                    op0=mybir.AluOpType.subtract, op1=mybir.AluOpType.mult)
                nc.vector.tensor_mul(O_sb, O_sb, ga_b)
                nc.vector.tensor_add(O_sb, O_sb, be_b)
                nc.sync.dma_start(
                    out=d["out_o"][qt * 128:(qt + 1) * 128, :], in_=O_sb)


_NC_CACHE = {}


def _get_nc(causal):
    if causal not in _NC_CACHE:
        _NC_CACHE[causal] = _build_nc(causal)
    return _NC_CACHE[causal]


def _qtiles(j):
    return sorted([j, 7 - j, 8 + j, 15 - j])


def kernel(query, key, value, mask, wq, bq, wk, bk, wv, bv, wo, bo,
           gamma, beta):
    query = np.ascontiguousarray(np.asarray(query, np.float32))
    key = np.ascontiguousarray(np.asarray(key, np.float32))
    value = np.ascontiguousarray(np.asarray(value, np.float32))
    mask = np.ascontiguousarray(np.asarray(mask, np.float32))
    wq = np.ascontiguousarray(np.asarray(wq, np.float32))
    wk = np.ascontiguousarray(np.asarray(wk, np.float32))
    wv = np.ascontiguousarray(np.asarray(wv, np.float32))
    wo = np.ascontiguousarray(np.asarray(wo, np.float32))
    bq = np.asarray(bq, np.float32)
    bk = np.asarray(bk, np.float32)
    bv = np.asarray(bv, np.float32)
    bo = np.asarray(bo, np.float32)
    gamma = np.asarray(gamma, np.float32)
    beta = np.asarray(beta, np.float32)

    causal_ref = np.triu(np.ones((S, S), np.float32), k=1)
    causal = all(np.array_equal(mask[b, 0], causal_ref) for b in range(B))

    nc = _get_nc(causal)
    in_maps = []
    for c in range(NCORES):
        b = c // 4
        j = c % 4
        tiles = _qtiles(j)
        qrows = np.concatenate(
            [np.arange(g * 128, (g + 1) * 128) for g in tiles])
        qs = query[b][qrows]
        if causal:
            mk = np.stack([
                mask[b, 0, tiles[t] * 128:(tiles[t] + 1) * 128,
                     t * 512:(t + 1) * 512]
                for t in range(NQT)]).astype(ml_dtypes.bfloat16)
        else:
            mk = mask[b, 0][qrows].astype(ml_dtypes.bfloat16)
        in_maps.append({
            "qT": np.ascontiguousarray(qs.T),
            "qn": np.ascontiguousarray(qs + bo[None, :]),
            "kT": np.ascontiguousarray(key[b].T),
            "vT": np.ascontiguousarray(value[b].T),
            "mk": np.ascontiguousarray(mk),
            "wq": wq, "wk": wk, "wv": wv, "wo": wo,
            "bq": bq, "bk": bk, "bv": bv,
            "ga": gamma, "be": beta,
        })

    r = run_bass_kernel_spmd(nc, in_maps, core_ids=list(range(NCORES)),
                             trace=TRACE)
    if TRACE:
        kernel.last_results = r

    out = np.empty((B, S, D), np.float32)
    attn = np.empty((B, H, S, S), np.float32)
    for c in range(NCORES):
        b = c // 4
        tiles = _qtiles(c % 4)
        for t, g in enumerate(tiles):
            out[b, g * 128:(g + 1) * 128, :] = \
                r.results[c]["out_o"][t * 128:(t + 1) * 128]
            attn[b, :, g * 128:(g + 1) * 128, :] = \
                r.results[c]["attn_o"][:, t * 128:(t + 1) * 128, :]
    return out, attn
